# revision 66
# baseline (speedup 1.0000x reference)
"""Trainium2 Bass kernel for nn_DistanceAwareSelfAttentionHead.

Math (reference):
    s  = sigmoid((edge_attr - ib) * im)                    [E]
    rk = Ek1 + s*dEk ; rq = Eq1 + s*dEq ; rv = Ev1 + s*dEv (per edge, rank-1 in s)
    k = x@Wk ; q = x@Wq ; v = x@Wv
    A  = 2 q k^T ; A[src,dst] += q[src].rk + k[dst].rq     (duplicate edges summed)
    P  = softmax(A / sqrt(512))
    M  = P v + segsum(P[src,dst] * rv, src)

Identities:
    summed bias of a cell = mu*(a1+b1) + Sd*(a2+b2)
        mu = cell multiplicity, Sd = sum of sigmoids over the cell's edges
        a1 = q.Ek1, a2 = q.dEk (per row), b1 = k.Eq1, b2 = k.dEq (per col)
    segsum(P*rv) row r = u1[r]*Ev1 + u2[r]*dEv, u1 = sum mu*P, u2 = sum Sd*P
    M = P@v = (P@x)@Wv  (v never materialized)
    softmax without max-subtraction (logits bounded), normalize at the end.

Sharding: rows of A/q/M split across 8 cores (512 rows each); k and the
x-resident matmuls replicated per core (no collectives). Host precomputes
dense per-(core,row-block) mu and Sd matrices (index/attr transforms only).

Per-core schedule (engines balanced; one PSUM pool spans both phases so
there is no drain barrier at the boundary; bulk DMA batched because every
dma_start costs ~625ns of serialized HWDGE descriptor generation):
  phase 1: kT (bf16 in, f32r out, weight-stationary half-sweeps over the
           8 PSUM banks), qT2, a12, b-rows via eqb^T@kT per half, then
           b-tables broadcast to 128 partitions by a DMA bounce through
           DRAM on the ACT HWDGE queue (no engine time, jumps the SP queue).
  phase 2 per row-block rb (software-pipelined, emission order = per-engine
           execution order):
           bias ts = (b1bc+a1)*mu + (b2bc+a2)*Sd via plain tensor_scalar/
           tensor_tensor (DVE 2x fast mode; the sd-mult and final add ride
           the Pool engine except for rb0 whose bias gates the pipeline);
           A chunk pairs: 2 q.k matmuls first (never wait on bias), ident-
           matmul seeds the bias last; exp on ACT with accum_out -> Z;
           u1/u2 = accum(mu*P), accum(Sd*P) fused stt halves on DVE;
           P^T tiles via PE transposes (ACT drains; PX matmuls lag one
           group to hide the copy); PX = P@x accumulated in PSUM;
           M = (PX)^T @ Wv; each rb's M-tail is deferred into the next
           rb's A-pair window so its latency overlaps.
"""

import sys

if "/opt/trn_rl_repo" not in sys.path:
    sys.path.insert(0, "/opt/trn_rl_repo")

from contextlib import ExitStack

import numpy as np

import concourse.bacc as bacc
import concourse.mybir as mybir
import concourse.tile as tile
from concourse.bass_utils import run_bass_kernel_spmd

try:
    import ml_dtypes

    BF16_NP = ml_dtypes.bfloat16
except Exception:  # pragma: no cover
    BF16_NP = None

# ---- problem constants (hardcoded per the harness contract) ----
N = 4096
FEAT = 512
HID = 256
NCORES = 8
RPC = N // NCORES  # 512 rows per core
RB = RPC // 128  # 4 row blocks
SCALE = 1.0 / np.sqrt(np.float32(FEAT))

F32 = mybir.dt.float32
F32R = mybir.dt.float32r
BF16 = mybir.dt.bfloat16

AF = mybir.ActivationFunctionType
OP = mybir.AluOpType
AX = mybir.AxisListType

_CACHE = {}
DEBUG_TAPS = False
QK_BF16 = True  # bf16 for the q/k pipeline (fallback: f32r)


def _build_nc(reps=1):
    nc = bacc.Bacc(
        "TRN2",
        target_bir_lowering=False,
        debug=False,
        enable_asserts=False,
        num_devices=NCORES,
    )
    d = {}

    def din(name, shape, dtype=F32):
        d[name] = nc.dram_tensor(name, shape, dtype, kind="ExternalInput").ap()

    QK = BF16 if QK_BF16 else F32R
    din("xt", [4, 128, N], QK)  # x^T, feat-major chunks
    din("xtloc", [4, 128, RPC], QK)  # local columns of x^T
    din("xb", [32, 128, FEAT], BF16)  # x node-major tiles (bf16)
    din("wk", [4, 128, HID], QK)
    din("wq2", [4, 128, HID], QK)  # 2*Wq
    din("wv", [4, 128, FEAT], BF16)  # Wv feat-in chunks
    din("ekbh", [2, 128, 2], F32R)  # [Ek1 | dEk] / 2  (chunked over HID)
    din("eqb", [2, 128, 2], F32R)  # [Eq1 | dEq]
    din("ev1bc", [128, FEAT], BF16)  # Ev1 broadcast
    din("devbc", [128, FEAT], BF16)  # dEv broadcast
    din("ident", [128, 128], BF16)
    din("mu", [RB, 128, N], BF16)  # dense cell multiplicity, 0 off-cell
    din("sd", [RB, 128, N], BF16)  # dense per-cell sum of sigmoids
    mout = nc.dram_tensor("mloc", [RPC, FEAT], F32, kind="ExternalOutput").ap()
    # DRAM bounce for partition-broadcasting the b-row tables
    brow_d = nc.dram_tensor("browd", [2, N], BF16, kind="Internal").ap()
    dbg = {}
    if DEBUG_TAPS:
        for nm, shape, dt in [
            ("dkt", [128, 2, N], F32),
            ("dqt2", [128, 2, RPC], F32),
            ("da12", [128, RB, 2], F32),
            ("db1", [128, N], BF16),
            ("db2", [128, N], BF16),
            ("dts", [128, N], BF16),
            ("dexpa", [128, N], BF16),
            ("dpxs", [128, FEAT], BF16),
            ("dsm", [128, 4], F32),
        ]:
            dbg[nm] = nc.dram_tensor(nm, shape, dt, kind="ExternalOutput").ap()

    with tile.TileContext(nc) as tc:
        with ExitStack() as ctx:
            cpool = ctx.enter_context(tc.tile_pool(name="consts", bufs=1))
            wk_t = cpool.tile([128, 4, HID], QK)
            wq2_t = cpool.tile([128, 4, HID], QK)
            wv_t = cpool.tile([128, 4, FEAT], BF16)
            ekbh_t = cpool.tile([128, 2, 2], F32R)
            eqb_t = cpool.tile([128, 2, 2], F32R)
            ident_t = cpool.tile([128, 128], BF16)
            ev1bc_t = cpool.tile([128, FEAT], BF16)
            devbc_t = cpool.tile([128, FEAT], BF16)
            xb_t = cpool.tile([128, 32, FEAT], BF16)

            # one DMA per tensor — each dma_start costs ~625ns of serialized
            # HWDGE descriptor generation, so small transfers must be batched.
            # Only wk gates the first matmuls; the rest go after xt (emitted
            # in _body) via this deferred hook.
            nc.sync.dma_start(wk_t[:], d["wk"][:].rearrange("c p h -> p c h"))

            def load_consts():
                nc.sync.dma_start(
                    wq2_t[:], d["wq2"][:].rearrange("c p h -> p c h")
                )
                nc.sync.dma_start(ident_t[:], d["ident"][:])
                nc.sync.dma_start(
                    ekbh_t[:], d["ekbh"][:].rearrange("c p h -> p c h")
                )
                nc.sync.dma_start(eqb_t[:], d["eqb"][:].rearrange("c p h -> p c h"))
                nc.sync.dma_start(wv_t[:], d["wv"][:].rearrange("c p h -> p c h"))
                nc.sync.dma_start(ev1bc_t[:], d["ev1bc"][:])
                nc.sync.dma_start(devbc_t[:], d["devbc"][:])

            def _body():
                with tc.tile_pool(name="mid", bufs=1) as pmid:
                    kt_t = pmid.tile([128, 2, N], F32R)
                    qt2_t = pmid.tile([128, 2, RPC], F32R)
                    b1bc_t = pmid.tile([128, N], BF16)
                    b2bc_t = pmid.tile([128, N], BF16)
                    a12_t = pmid.tile([128, RB, 2], F32)
                    brow_t = pmid.tile([2, N], BF16)

                    dmat_ctx = ExitStack()
                    dmp = dmat_ctx.enter_context(tc.tile_pool(name="dmat", bufs=2))
                    mu_l, sd_l = [], []

                    def load_rb(rb):
                        m = dmp.tile([128, N], BF16, tag="mu", name="mu")
                        s = dmp.tile([128, N], BF16, tag="sd", name="sd")
                        nc.sync.dma_start(m[:], d["mu"][rb])
                        nc.sync.dma_start(s[:], d["sd"][rb])
                        mu_l.append(m)
                        sd_l.append(s)

                    ps1 = dmat_ctx.enter_context(
                        tc.tile_pool(name="ps", bufs=1, space="PSUM")
                    )
                    ps2 = ps1
                    # ---------- phase 1: kT, qT2, a12, b tables ---------------
                    with tc.tile_pool(name="ph1", bufs=1) as p1:
                        xt_t = p1.tile([128, 4, N], QK)
                        xtloc_t = p1.tile([128, 4, RPC], QK)
                        # DMA order = criticality: xt feeds the k matmuls
                        # that gate everything else; h-outer so the first
                        # column-half of every kc chunk lands first
                        # split xt across the SP and ACT HWDGE queues so
                        # descriptor generation runs in parallel
                        for kc in range(4):
                            eng = nc.sync if kc % 2 == 0 else nc.scalar
                            eng.dma_start(
                                xt_t[:, kc, 0:2048], d["xt"][kc][:, 0:2048]
                            )
                        nc.sync.dma_start(
                            xtloc_t[:], d["xtloc"][:].rearrange("c p f -> p c f")
                        )
                        for kc in range(4):
                            eng = nc.sync if kc % 2 == 0 else nc.scalar
                            eng.dma_start(
                                xt_t[:, kc, 2048:N], d["xt"][kc][:, 2048:N]
                            )
                        load_consts()
                        load_rb(0)
                        # x node-major tiles for the P@x matmuls (bf16)
                        for g in range(8):
                            nc.sync.dma_start(
                                xb_t[:, 4 * g : 4 * g + 4, :],
                                d["xb"][4 * g : 4 * g + 4].rearrange(
                                    "t p f -> p t f"
                                ),
                            )
                        load_rb(1)

                        def k_half(half):
                            for hg in range(2):
                                # weight-stationary half-sweep: 4 weight
                                # loads, 4 wide matmuls each, accum over kc
                                kbank = [
                                    ps1.tile(
                                        [128, 512], F32, tag=f"kb{i}",
                                        name=f"kb{i}", bufs=2,
                                    )
                                    for i in range(4)
                                ]
                                for kc in range(4):
                                    for i in range(4):
                                        n = 4 * half + i
                                        nc.tensor.matmul(
                                            kbank[i][:],
                                            wk_t[:, kc, hg * 128 : (hg + 1) * 128],
                                            xt_t[:, kc, n * 512 : (n + 1) * 512],
                                            start=(kc == 0),
                                            stop=(kc == 3),
                                        )
                                # drain to kt (f32r), spread across engines
                                for i in range(4):
                                    n = 4 * half + i
                                    dst = kt_t[:, hg, n * 512 : (n + 1) * 512]
                                    if i % 2 == 0:
                                        nc.scalar.copy(dst, kbank[i][:])
                                    else:
                                        nc.vector.tensor_copy(dst, kbank[i][:])
                            # b rows for this half's chunks (needs both hg)
                            for i in range(4):
                                n = 4 * half + i
                                bps = ps1.tile(
                                    [128, 512], F32, tag=f"kb{i}", name="bps",
                                    bufs=2,
                                )
                                for hg in range(2):
                                    nc.tensor.matmul(
                                        bps[0:2, :],
                                        eqb_t[:, hg, :],
                                        kt_t[:, hg, n * 512 : (n + 1) * 512],
                                        start=(hg == 0),
                                        stop=(hg == 1),
                                    )
                                nc.vector.tensor_copy(
                                    brow_t[:, n * 512 : (n + 1) * 512], bps[0:2, :]
                                )
                            # bounce this half's b rows through DRAM to
                            # partition-broadcast them; ACT HWDGE queue so
                            # they jump ahead of the bulk SP DMA traffic
                            cols = slice(half * 2048, (half + 1) * 2048)
                            nc.scalar.dma_start(brow_d[:, cols], brow_t[:, cols])
                            nc.scalar.dma_start(
                                b1bc_t[:, cols],
                                brow_d[0:1, cols].broadcast_to([128, 2048]),
                            )
                            nc.scalar.dma_start(
                                b2bc_t[:, cols],
                                brow_d[1:2, cols].broadcast_to([128, 2048]),
                            )

                        def q_and_a12():
                            # qT2 (2*q, hid-major, local rows)
                            for hg in range(2):
                                qps = ps1.tile(
                                    [128, 512], F32, tag="kb0", name="qps", bufs=2
                                )
                                for kc in range(4):
                                    nc.tensor.matmul(
                                        qps[:],
                                        wq2_t[:, kc, hg * 128 : (hg + 1) * 128],
                                        xtloc_t[:, kc, :],
                                        start=(kc == 0),
                                        stop=(kc == 3),
                                    )
                                nc.vector.tensor_copy(qt2_t[:, hg, :], qps[:])
                            # a12: per-row (a1|a2) = q.Ek1 | q.dEk
                            for rb in range(RB):
                                aps = ps1.tile(
                                    [128, 512], F32, tag="kb1", name="aps", bufs=2
                                )
                                for hg in range(2):
                                    nc.tensor.matmul(
                                        aps[:, 0:2],
                                        qt2_t[:, hg, rb * 128 : (rb + 1) * 128],
                                        ekbh_t[:, hg, :],
                                        start=(hg == 0),
                                        stop=(hg == 1),
                                    )
                                nc.vector.tensor_copy(a12_t[:, rb, :], aps[:, 0:2])

                        k_half(0)
                        q_and_a12()
                        k_half(1)

                    # ---------- phase 2: bias + A + exp + PX + M --------------
                    with dmat_ctx, tc.tile_pool(name="work", bufs=1) as wp:

                        ts_l = []

                        def bias_rb(rb):
                            # ts = (b1bc + a1)*mu + (b2bc + a2)*sd, emitted in
                            # column halves so rb0's first A-chunk seeds only
                            # wait on the first half of the table bounce.
                            # plain tensor_scalar / tensor_tensor get the DVE
                            # 2x fast mode; scalar_tensor_tensor does not.
                            # rb0 runs entirely on DVE: its bias gates the
                            # whole phase-2 pipeline, and the Pool engine's
                            # software ops are ~4x slower per element.
                            t1 = wp.tile([128, 2048], BF16, tag="t1", bufs=1)
                            ts = wp.tile([128, N], BF16, tag="ts", bufs=2)
                            eng = nc.vector if rb == 0 else nc.gpsimd
                            for cols in (slice(0, 2048), slice(2048, N)):
                                nc.vector.tensor_scalar(
                                    t1[:], b1bc_t[:, cols],
                                    a12_t[:, rb, 0:1], None, OP.add,
                                )
                                nc.vector.tensor_tensor(
                                    t1[:], t1[:], mu_l[rb][:, cols], OP.mult
                                )
                                nc.vector.tensor_scalar(
                                    ts[:, cols], b2bc_t[:, cols],
                                    a12_t[:, rb, 1:2], None, OP.add,
                                )
                                eng.tensor_tensor(
                                    ts[:, cols], ts[:, cols], sd_l[rb][:, cols],
                                    OP.mult,
                                )
                                eng.tensor_tensor(
                                    ts[:, cols], ts[:, cols], t1[:], OP.add
                                )
                            ts_l.append(ts)

                        bias_rb(0)
                        if DEBUG_TAPS:
                            nc.sync.dma_start(dbg["dkt"][:, :, :], kt_t[:].bitcast(F32))
                            nc.sync.dma_start(dbg["dqt2"][:, :, :], qt2_t[:].bitcast(F32))
                            nc.sync.dma_start(dbg["da12"][:, :, :], a12_t[:])
                            nc.sync.dma_start(dbg["db1"][:, :], b1bc_t[:])
                            nc.sync.dma_start(dbg["db2"][:, :], b2bc_t[:])
                            nc.sync.dma_start(dbg["dts"][:, :], ts_l[0][:])

                        tail_prev = None
                        for rb in range(RB):
                            ts = ts_l[rb]
                            expa = wp.tile([128, N], BF16, tag="expa", bufs=2)
                            zacc = wp.tile([128, 8], F32, tag="zacc", bufs=2)
                            wscr = wp.tile([128, N], BF16, tag="wscr", bufs=1)
                            sm = wp.tile([128, 4], F32, tag="sm", bufs=2)
                            pxa = ps2.tile([128, FEAT], F32, tag="kb2", bufs=2, name="pxa")
                            pxb = ps2.tile([128, FEAT], F32, tag="kb2", bufs=2, name="pxb")
                            abank_of = {}

                            def a_pair(p):
                                # chunks 2p, 2p+1: bias seed + 2 qk matmuls
                                n0, n1 = 2 * p, 2 * p + 1
                                ab = [
                                    ps2.tile([128, 512], F32, tag=f"kb{i}",
                                             bufs=2, name=f"ab{i}")
                                    for i in range(2)
                                ]
                                abank_of[n0] = ab[0]
                                abank_of[n1] = ab[1]
                                # qk first so A matmuls never wait on the
                                # DVE bias chain; the bias seed lands last
                                for hg in range(2):
                                    for i, n in ((0, n0), (1, n1)):
                                        nc.tensor.matmul(
                                            ab[i][:],
                                            qt2_t[:, hg, rb * 128 : (rb + 1) * 128],
                                            kt_t[:, hg, n * 512 : (n + 1) * 512],
                                            start=(hg == 0), stop=False,
                                            skip_group_check=True,
                                        )
                                for i, n in ((0, n0), (1, n1)):
                                    nc.tensor.matmul(
                                        ab[i][:], ident_t[:],
                                        ts[:, n * 512 : (n + 1) * 512],
                                        start=False, stop=True,
                                        skip_group_check=True,
                                    )

                            def exp_chunk(n):
                                nc.scalar.activation(
                                    expa[:, n * 512 : (n + 1) * 512],
                                    abank_of.pop(n)[:],
                                    AF.Exp,
                                    scale=float(SCALE),
                                    accum_out=zacc[:, n : n + 1],
                                )

                            u1c = wp.tile([128, 2], F32, tag="u1c", bufs=2)
                            u2c = wp.tile([128, 2], F32, tag="u2c", bufs=2)

                            def u_half(h):
                                # fused multiply+row-accumulate on DVE (the
                                # plain TensorReduce runs at 1x, this is the
                                # same cost with the reduce folded in)
                                cols = slice(h * 2048, (h + 1) * 2048)
                                nc.vector.scalar_tensor_tensor(
                                    wscr[:, cols], mu_l[rb][:, cols], 1.0,
                                    expa[:, cols], OP.mult, OP.mult,
                                    accum_out=u1c[:, h : h + 1],
                                )
                                nc.vector.scalar_tensor_tensor(
                                    wscr[:, cols], sd_l[rb][:, cols], 1.0,
                                    expa[:, cols], OP.mult, OP.mult,
                                    accum_out=u2c[:, h : h + 1],
                                )

                            pt_of = {}

                            def tp_t(g):
                                # transpose P chunk g into PT; drain on DVE
                                # for early groups (ACT still busy with exps)
                                # and on ACT for late ones (DVE busy with u)
                                tp = ps2.tile([128, 1024], BF16, tag="kb3", bufs=2, name="tp")
                                pt = wp.tile([128, 512], BF16, tag="pt", bufs=2)
                                pt_of[g] = pt
                                for j in range(4):
                                    c = 4 * g + j
                                    nc.tensor.transpose(
                                        tp[:, j * 128 : (j + 1) * 128],
                                        expa[:, c * 128 : (c + 1) * 128],
                                        ident_t[:],
                                    )
                                nc.scalar.copy(pt[:], tp[:, 0:512])

                            def tp_x(g):
                                # 4 PX accumulation matmuls vs x tiles; PX is
                                # split into two half-accumulations so the
                                # M-tail for the first half overlaps groups
                                # 4-7 instead of serializing after group 7
                                px = pxa if g < 4 else pxb
                                pt = pt_of.pop(g)
                                for j in range(4):
                                    c = 4 * g + j
                                    nc.tensor.matmul(
                                        px[:],
                                        pt[:, j * 128 : (j + 1) * 128],
                                        xb_t[:, c, :],
                                        start=(c % 16 == 0),
                                        stop=(c % 16 == 15),
                                        skip_group_check=True,
                                    )

                            def half_tail(px, mm, first):
                                pxs = wp.tile([128, FEAT], BF16, tag="pxs",
                                              bufs=2)
                                nc.scalar.copy(pxs[:], px[:])
                                tp2 = ps2.tile([128, 1024], BF16, tag="kb3",
                                               bufs=2, name="tp2")
                                pxt = wp.tile([128, 512], BF16, tag="pxt",
                                              bufs=2)
                                for j in range(4):
                                    nc.tensor.transpose(
                                        tp2[:, j * 128 : (j + 1) * 128],
                                        pxs[:, j * 128 : (j + 1) * 128],
                                        ident_t[:],
                                    )
                                nc.scalar.copy(pxt[:], tp2[:, 0:512])
                                for j in range(4):
                                    nc.tensor.matmul(
                                        mm[:],
                                        pxt[:, j * 128 : (j + 1) * 128],
                                        wv_t[:, j, :],
                                        start=(first and j == 0),
                                        stop=((not first) and j == 3),
                                        skip_group_check=True,
                                    )

                            def mk_tail(rb, pxb, mm, zacc, sm, expa):
                                def tail():
                                    # Z and u scaling
                                    nc.vector.tensor_reduce(
                                        sm[:, 0:1], zacc[:], AX.X, OP.add
                                    )
                                    nc.vector.reciprocal(sm[:, 1:2], sm[:, 0:1])
                                    nc.vector.tensor_scalar(
                                        sm[:, 2:3], sm[:, 2:3], sm[:, 1:2],
                                        None, OP.mult,
                                    )
                                    nc.vector.tensor_scalar(
                                        sm[:, 3:4], sm[:, 3:4], sm[:, 1:2],
                                        None, OP.mult,
                                    )
                                    # M tail, second PX half
                                    half_tail(pxb, mm, first=False)
                                    # mf = mm*rz + u1*rz*Ev1 + u2*rz*dEv
                                    mcp = wp.tile(
                                        [128, FEAT], F32, tag="mcp", bufs=1
                                    )
                                    mf = wp.tile([128, FEAT], F32, tag="mf",
                                                 bufs=1)
                                    nc.scalar.activation(
                                        mcp[:], mm[:], AF.Identity,
                                        scale=sm[:, 1:2],
                                    )
                                    nc.vector.scalar_tensor_tensor(
                                        mcp[:], ev1bc_t[:], sm[:, 2:3], mcp[:],
                                        OP.mult, OP.add,
                                    )
                                    nc.vector.scalar_tensor_tensor(
                                        mf[:], devbc_t[:], sm[:, 3:4], mcp[:],
                                        OP.mult, OP.add,
                                    )
                                    nc.sync.dma_start(
                                        mout[rb * 128 : (rb + 1) * 128, :], mf[:]
                                    )
                                    if DEBUG_TAPS and rb == 0:
                                        nc.sync.dma_start(
                                            dbg["dexpa"][:, :], expa[:]
                                        )
                                        nc.sync.dma_start(dbg["dpxs"][:, :], pxs[:])
                                        nc.sync.dma_start(dbg["dsm"][:, :], sm[:])

                                return tail

                            # pipelined emission: transpose groups lag the A
                            # pairs by two pairs, PX matmuls lag their
                            # transposes by one group (hides the Pool copy),
                            # and the previous rb's M-tail fills this rb's
                            # A-pair window
                            a_pair(0)
                            a_pair(1)
                            if tail_prev is not None:
                                tail_prev()
                            exp_chunk(0)
                            exp_chunk(1)
                            exp_chunk(2)
                            exp_chunk(3)
                            u_half(0)
                            a_pair(2)
                            exp_chunk(4)
                            exp_chunk(5)
                            tp_t(0)
                            tp_t(1)
                            tp_x(0)
                            a_pair(3)
                            exp_chunk(6)
                            exp_chunk(7)
                            tp_t(2)
                            tp_x(1)
                            tp_t(3)
                            tp_x(2)
                            u_half(1)
                            # prefetch + bias for rb+1 before this rb's tail
                            if rb + 2 < RB:
                                load_rb(rb + 2)
                            if rb + 1 < RB:
                                bias_rb(rb + 1)
                            tp_t(4)
                            tp_x(3)
                            tp_t(5)
                            tp_x(4)
                            mm = ps2.tile([128, FEAT], F32, tag="kb2",
                                          bufs=2, name="mm")
                            half_tail(pxa, mm, first=True)
                            tp_t(6)
                            tp_x(5)
                            tp_t(7)
                            tp_x(6)
                            tp_x(7)
                            nc.vector.tensor_reduce(
                                sm[:, 2:3], u1c[:], AX.X, OP.add
                            )
                            nc.vector.tensor_reduce(
                                sm[:, 3:4], u2c[:], AX.X, OP.add
                            )
                            tail_prev = mk_tail(rb, pxb, mm, zacc, sm, expa)
                        tail_prev()

            if reps > 1:
                with tc.For_i(0, reps, 1):
                    _body()
            else:
                _body()

    nc.compile()
    return nc


def _prep(inputs):
    x = np.asarray(inputs["x"], np.float32)
    ei = np.asarray(inputs["edge_index"]).astype(np.int64)
    ea = np.asarray(inputs["edge_attr"], np.float32).reshape(-1)
    Wk = np.asarray(inputs["Wk"], np.float32)
    Wq = np.asarray(inputs["Wq"], np.float32)
    Wv = np.asarray(inputs["Wv"], np.float32)
    Ek = np.asarray(inputs["Ek"], np.float32)
    Eq = np.asarray(inputs["Eq"], np.float32)
    Ev = np.asarray(inputs["Ev"], np.float32)
    ib = float(np.asarray(inputs["idx_bias"]).reshape(()))
    im = float(np.asarray(inputs["idx_mult"]).reshape(()))

    assert BF16_NP is not None, "ml_dtypes needed"
    bf = BF16_NP

    src, dst = ei[0], ei[1]
    s_all = 1.0 / (1.0 + np.exp(-(ea - ib) * im))
    # dense per-(core, row-block) multiplicity and sigmoid-sum matrices
    flat = src * N + dst
    mu_f = np.zeros(N * N, np.float32)
    sd_f = np.zeros(N * N, np.float32)
    np.add.at(mu_f, flat, 1.0)
    np.add.at(sd_f, flat, s_all)
    mu_d = mu_f.reshape(NCORES, RB, 128, N).astype(bf)
    sd_d = sd_f.reshape(NCORES, RB, 128, N).astype(bf)

    qk = bf if QK_BF16 else np.float32
    xT = np.ascontiguousarray(x.T)  # [FEAT, N]
    shared = {
        "xt": np.ascontiguousarray(xT.reshape(4, 128, N).astype(qk)),
        "xb": np.ascontiguousarray(x.reshape(32, 128, FEAT).astype(bf)),
        "wk": np.ascontiguousarray(Wk.reshape(4, 128, HID).astype(qk)),
        "wq2": np.ascontiguousarray((2.0 * Wq).reshape(4, 128, HID).astype(qk)),
        "wv": np.ascontiguousarray(Wv.reshape(4, 128, FEAT).astype(bf)),
        "ekbh": np.ascontiguousarray(
            (0.5 * np.stack([Ek[1], Ek[0] - Ek[1]], axis=1))
            .reshape(2, 128, 2)
        ),
        "eqb": np.ascontiguousarray(
            np.stack([Eq[1], Eq[0] - Eq[1]], axis=1).reshape(2, 128, 2)
        ),
        "ev1bc": np.ascontiguousarray(
            np.broadcast_to(Ev[1], (128, FEAT)).astype(bf)
        ),
        "devbc": np.ascontiguousarray(
            np.broadcast_to(Ev[0] - Ev[1], (128, FEAT)).astype(bf)
        ),
        "ident": np.eye(128, dtype=np.float32).astype(bf),
    }
    in_maps = []
    for cc in range(NCORES):
        m = dict(shared)
        m["xtloc"] = np.ascontiguousarray(
            xT[:, cc * RPC : (cc + 1) * RPC].reshape(4, 128, RPC).astype(qk)
        )
        m["mu"] = mu_d[cc]
        m["sd"] = sd_d[cc]
        in_maps.append(m)
    return in_maps


def get_nc(reps=1):
    if reps not in _CACHE:
        _CACHE[reps] = _build_nc(reps)
    return _CACHE[reps]


def kernel(**inputs) -> np.ndarray:
    nc = get_nc()
    in_maps = _prep(inputs)
    res = run_bass_kernel_spmd(nc, in_maps, list(range(NCORES)))
    return np.concatenate(
        [res.results[cc]["mloc"] for cc in range(NCORES)], axis=0
    ).astype(np.float32)


# revision 72
# speedup vs baseline: 1.0015x; 1.0015x over previous
"""Trainium2 Bass kernel for nn_DistanceAwareSelfAttentionHead.

Math (reference):
    s  = sigmoid((edge_attr - ib) * im)                    [E]
    rk = Ek1 + s*dEk ; rq = Eq1 + s*dEq ; rv = Ev1 + s*dEv (per edge, rank-1 in s)
    k = x@Wk ; q = x@Wq ; v = x@Wv
    A  = 2 q k^T ; A[src,dst] += q[src].rk + k[dst].rq     (duplicate edges summed)
    P  = softmax(A / sqrt(512))
    M  = P v + segsum(P[src,dst] * rv, src)

Identities:
    summed bias of a cell = mu*(a1+b1) + Sd*(a2+b2)
        mu = cell multiplicity, Sd = sum of sigmoids over the cell's edges
        a1 = q.Ek1, a2 = q.dEk (per row), b1 = k.Eq1, b2 = k.dEq (per col)
    segsum(P*rv) row r = u1[r]*Ev1 + u2[r]*dEv, u1 = sum mu*P, u2 = sum Sd*P
    M = P@v = (P@x)@Wv  (v never materialized)
    softmax without max-subtraction (logits bounded), normalize at the end.

Sharding: rows of A/q/M split across 8 cores (512 rows each); k and the
x-resident matmuls replicated per core (no collectives). Host precomputes
dense per-(core,row-block) mu and Sd matrices (index/attr transforms only).

Per-core schedule (engines balanced; one PSUM pool spans both phases so
there is no drain barrier at the boundary; bulk DMA batched because every
dma_start costs ~625ns of serialized HWDGE descriptor generation):
  phase 1: kT (bf16 in, f32r out, weight-stationary half-sweeps over the
           8 PSUM banks), qT2, a12, b-rows via eqb^T@kT per half, then
           b-tables broadcast to 128 partitions by a DMA bounce through
           DRAM on the ACT HWDGE queue (no engine time, jumps the SP queue).
  phase 2 per row-block rb (software-pipelined, emission order = per-engine
           execution order):
           bias ts = (b1bc+a1)*mu + (b2bc+a2)*Sd via plain tensor_scalar/
           tensor_tensor (DVE 2x fast mode; the sd-mult and final add ride
           the Pool engine except for rb0 whose bias gates the pipeline);
           A chunk pairs: 2 q.k matmuls first (never wait on bias), ident-
           matmul seeds the bias last; exp on ACT with accum_out -> Z;
           u1/u2 = accum(mu*P), accum(Sd*P) fused stt halves on DVE;
           P^T tiles via PE transposes (ACT drains; PX matmuls lag one
           group to hide the copy); PX = P@x accumulated in PSUM;
           M = (PX)^T @ Wv; each rb's M-tail is deferred into the next
           rb's A-pair window so its latency overlaps.
"""

import sys

if "/opt/trn_rl_repo" not in sys.path:
    sys.path.insert(0, "/opt/trn_rl_repo")

from contextlib import ExitStack

import numpy as np

import concourse.bacc as bacc
import concourse.mybir as mybir
import concourse.tile as tile
from concourse.bass_utils import run_bass_kernel_spmd

try:
    import ml_dtypes

    BF16_NP = ml_dtypes.bfloat16
except Exception:  # pragma: no cover
    BF16_NP = None

# ---- problem constants (hardcoded per the harness contract) ----
N = 4096
FEAT = 512
HID = 256
NCORES = 8
RPC = N // NCORES  # 512 rows per core
RB = RPC // 128  # 4 row blocks
SCALE = 1.0 / np.sqrt(np.float32(FEAT))

F32 = mybir.dt.float32
F32R = mybir.dt.float32r
BF16 = mybir.dt.bfloat16

AF = mybir.ActivationFunctionType
OP = mybir.AluOpType
AX = mybir.AxisListType

_CACHE = {}
DEBUG_TAPS = False
QK_BF16 = True  # bf16 for the q/k pipeline (fallback: f32r)


def _build_nc(reps=1):
    nc = bacc.Bacc(
        "TRN2",
        target_bir_lowering=False,
        debug=False,
        enable_asserts=False,
        num_devices=NCORES,
    )
    d = {}

    def din(name, shape, dtype=F32):
        d[name] = nc.dram_tensor(name, shape, dtype, kind="ExternalInput").ap()

    QK = BF16 if QK_BF16 else F32R
    din("xt", [4, 128, N], QK)  # x^T, feat-major chunks
    din("xtloc", [4, 128, RPC], QK)  # local columns of x^T
    din("xb", [32, 128, FEAT], BF16)  # x node-major tiles (bf16)
    din("wk", [4, 128, HID], QK)
    din("wq2", [4, 128, HID], QK)  # 2*Wq
    din("wv", [4, 128, FEAT], BF16)  # Wv feat-in chunks
    din("wba", [4, 128, 2], QK)  # Wq @ [Ek1 | dEk]  (feat chunks)
    din("wbq", [4, 128, 2], QK)  # Wk @ [Eq1 | dEq]  (feat chunks)
    din("ev1bc", [128, FEAT], BF16)  # Ev1 broadcast
    din("devbc", [128, FEAT], BF16)  # dEv broadcast
    din("ident", [128, 128], BF16)
    din("mu", [RB, 128, N], BF16)  # dense cell multiplicity, 0 off-cell
    din("sd", [RB, 128, N], BF16)  # dense per-cell sum of sigmoids
    mout = nc.dram_tensor("mloc", [RPC, FEAT], F32, kind="ExternalOutput").ap()
    # DRAM bounce for partition-broadcasting the b-row tables
    brow_d = nc.dram_tensor("browd", [2, N], BF16, kind="Internal").ap()
    dbg = {}
    if DEBUG_TAPS:
        for nm, shape, dt in [
            ("dkt", [128, 2, N], F32),
            ("dqt2", [128, 2, RPC], F32),
            ("da12", [128, RB, 2], F32),
            ("db1", [128, N], BF16),
            ("db2", [128, N], BF16),
            ("dts", [128, N], BF16),
            ("dexpa", [128, N], BF16),
            ("dpxs", [128, FEAT], BF16),
            ("dsm", [128, 4], F32),
        ]:
            dbg[nm] = nc.dram_tensor(nm, shape, dt, kind="ExternalOutput").ap()

    with tile.TileContext(nc) as tc:
        with ExitStack() as ctx:
            cpool = ctx.enter_context(tc.tile_pool(name="consts", bufs=1))
            wk_t = cpool.tile([128, 4, HID], QK)
            wq2_t = cpool.tile([128, 4, HID], QK)
            wv_t = cpool.tile([128, 4, FEAT], BF16)
            wba_t = cpool.tile([128, 4, 2], QK)
            wbq_t = cpool.tile([128, 4, 2], QK)
            ident_t = cpool.tile([128, 128], BF16)
            ev1bc_t = cpool.tile([128, FEAT], BF16)
            devbc_t = cpool.tile([128, FEAT], BF16)
            xb_t = cpool.tile([128, 32, FEAT], BF16)

            # one DMA per tensor — each dma_start costs ~625ns of serialized
            # HWDGE descriptor generation, so small transfers must be batched.
            # Only wk gates the first matmuls; the rest go after xt (emitted
            # in _body) via this deferred hook.
            nc.sync.dma_start(wk_t[:], d["wk"][:].rearrange("c p h -> p c h"))

            def load_consts():
                nc.sync.dma_start(
                    wq2_t[:], d["wq2"][:].rearrange("c p h -> p c h")
                )
                nc.sync.dma_start(ident_t[:], d["ident"][:])
                nc.sync.dma_start(
                    wba_t[:], d["wba"][:].rearrange("c p h -> p c h")
                )
                nc.sync.dma_start(wbq_t[:], d["wbq"][:].rearrange("c p h -> p c h"))
                nc.sync.dma_start(wv_t[:], d["wv"][:].rearrange("c p h -> p c h"))
                nc.sync.dma_start(ev1bc_t[:], d["ev1bc"][:])
                nc.sync.dma_start(devbc_t[:], d["devbc"][:])

            def _body():
                with tc.tile_pool(name="mid", bufs=1) as pmid:
                    kt_t = pmid.tile([128, 2, N], F32R)
                    qt2_t = pmid.tile([128, 2, RPC], F32R)
                    b1bc_t = pmid.tile([128, N], BF16)
                    b2bc_t = pmid.tile([128, N], BF16)
                    a12_t = pmid.tile([128, RB, 2], F32)
                    brow_t = pmid.tile([2, N], BF16)

                    dmat_ctx = ExitStack()
                    dmp = dmat_ctx.enter_context(tc.tile_pool(name="dmat", bufs=2))
                    mu_l, sd_l = [], []

                    def load_rb(rb):
                        m = dmp.tile([128, N], BF16, tag="mu", name="mu")
                        s = dmp.tile([128, N], BF16, tag="sd", name="sd")
                        nc.sync.dma_start(m[:], d["mu"][rb])
                        nc.sync.dma_start(s[:], d["sd"][rb])
                        mu_l.append(m)
                        sd_l.append(s)

                    ps1 = dmat_ctx.enter_context(
                        tc.tile_pool(name="ps", bufs=1, space="PSUM")
                    )
                    ps2 = ps1
                    # ---------- phase 1: kT, qT2, a12, b tables ---------------
                    with tc.tile_pool(name="ph1", bufs=1) as p1:
                        xt_t = p1.tile([128, 4, N], QK)
                        xtloc_t = p1.tile([128, 4, RPC], QK)
                        # DMA order = criticality: xt feeds the k matmuls
                        # that gate everything else; h-outer so the first
                        # column-half of every kc chunk lands first
                        # split xt across the SP and ACT HWDGE queues so
                        # descriptor generation runs in parallel
                        for kc in range(4):
                            eng = nc.sync if kc % 2 == 0 else nc.scalar
                            eng.dma_start(
                                xt_t[:, kc, 0:2048], d["xt"][kc][:, 0:2048]
                            )
                        nc.sync.dma_start(
                            xtloc_t[:], d["xtloc"][:].rearrange("c p f -> p c f")
                        )
                        for kc in range(4):
                            eng = nc.sync if kc % 2 == 0 else nc.scalar
                            eng.dma_start(
                                xt_t[:, kc, 2048:N], d["xt"][kc][:, 2048:N]
                            )
                        load_consts()
                        load_rb(0)
                        # x node-major tiles for the P@x matmuls (bf16)
                        for g in range(8):
                            nc.sync.dma_start(
                                xb_t[:, 4 * g : 4 * g + 4, :],
                                d["xb"][4 * g : 4 * g + 4].rearrange(
                                    "t p f -> p t f"
                                ),
                            )
                        load_rb(1)

                        def k_half(half):
                            for hg in range(2):
                                # weight-stationary half-sweep: 4 weight
                                # loads, 4 wide matmuls each, accum over kc
                                kbank = [
                                    ps1.tile(
                                        [128, 512], F32, tag=f"kb{i}",
                                        name=f"kb{i}", bufs=2,
                                    )
                                    for i in range(4)
                                ]
                                for kc in range(4):
                                    for i in range(4):
                                        n = 4 * half + i
                                        nc.tensor.matmul(
                                            kbank[i][:],
                                            wk_t[:, kc, hg * 128 : (hg + 1) * 128],
                                            xt_t[:, kc, n * 512 : (n + 1) * 512],
                                            start=(kc == 0),
                                            stop=(kc == 3),
                                        )
                                # drain to kt (f32r), spread across engines
                                for i in range(4):
                                    n = 4 * half + i
                                    dst = kt_t[:, hg, n * 512 : (n + 1) * 512]
                                    if i % 2 == 0:
                                        nc.scalar.copy(dst, kbank[i][:])
                                    else:
                                        nc.vector.tensor_copy(dst, kbank[i][:])
                            # b rows straight from x^T (host premultiplied
                            # wbq = Wk @ [Eq1|dEq])
                            for i in range(4):
                                n = 4 * half + i
                                bps = ps1.tile(
                                    [128, 512], F32, tag=f"kb{i}", name="bps",
                                    bufs=2,
                                )
                                for kc in range(4):
                                    nc.tensor.matmul(
                                        bps[0:2, :],
                                        wbq_t[:, kc, :],
                                        xt_t[:, kc, n * 512 : (n + 1) * 512],
                                        start=(kc == 0),
                                        stop=(kc == 3),
                                    )
                                nc.vector.tensor_copy(
                                    brow_t[:, n * 512 : (n + 1) * 512], bps[0:2, :]
                                )
                            cols = slice(half * 2048, (half + 1) * 2048)
                            nc.scalar.dma_start(brow_d[:, cols], brow_t[:, cols])
                            nc.scalar.dma_start(
                                b1bc_t[:, cols],
                                brow_d[0:1, cols].broadcast_to([128, 2048]),
                            )
                            nc.scalar.dma_start(
                                b2bc_t[:, cols],
                                brow_d[1:2, cols].broadcast_to([128, 2048]),
                            )

                        def q_and_a12():
                            # qT2 (2*q, hid-major, local rows)
                            for hg in range(2):
                                qps = ps1.tile(
                                    [128, 512], F32, tag="kb0", name="qps", bufs=2
                                )
                                for kc in range(4):
                                    nc.tensor.matmul(
                                        qps[:],
                                        wq2_t[:, kc, hg * 128 : (hg + 1) * 128],
                                        xtloc_t[:, kc, :],
                                        start=(kc == 0),
                                        stop=(kc == 3),
                                    )
                                nc.vector.tensor_copy(qt2_t[:, hg, :], qps[:])
                            # a12: per-row (a1|a2) = x_loc @ (Wq [Ek1|dEk])
                            for rb in range(RB):
                                aps = ps1.tile(
                                    [128, 512], F32, tag="kb1", name="aps", bufs=2
                                )
                                for kc in range(4):
                                    nc.tensor.matmul(
                                        aps[:, 0:2],
                                        xtloc_t[:, kc, rb * 128 : (rb + 1) * 128],
                                        wba_t[:, kc, :],
                                        start=(kc == 0),
                                        stop=(kc == 3),
                                    )
                                nc.vector.tensor_copy(a12_t[:, rb, :], aps[:, 0:2])

                        k_half(0)
                        q_and_a12()
                        k_half(1)

                    # ---------- phase 2: bias + A + exp + PX + M --------------
                    with dmat_ctx, tc.tile_pool(name="work", bufs=1) as wp:

                        ts_l = []

                        def bias_rb(rb):
                            # ts = (b1bc + a1)*mu + (b2bc + a2)*sd, emitted in
                            # column halves so rb0's first A-chunk seeds only
                            # wait on the first half of the table bounce.
                            # plain tensor_scalar / tensor_tensor get the DVE
                            # 2x fast mode; scalar_tensor_tensor does not.
                            # rb0 runs entirely on DVE: its bias gates the
                            # whole phase-2 pipeline, and the Pool engine's
                            # software ops are ~4x slower per element.
                            t1 = wp.tile([128, 2048], BF16, tag="t1", bufs=1)
                            ts = wp.tile([128, N], BF16, tag="ts", bufs=2)
                            eng = nc.vector if rb == 0 else nc.gpsimd
                            for cols in (slice(0, 2048), slice(2048, N)):
                                nc.vector.tensor_scalar(
                                    t1[:], b1bc_t[:, cols],
                                    a12_t[:, rb, 0:1], None, OP.add,
                                )
                                nc.vector.tensor_tensor(
                                    t1[:], t1[:], mu_l[rb][:, cols], OP.mult
                                )
                                nc.vector.tensor_scalar(
                                    ts[:, cols], b2bc_t[:, cols],
                                    a12_t[:, rb, 1:2], None, OP.add,
                                )
                                eng.tensor_tensor(
                                    ts[:, cols], ts[:, cols], sd_l[rb][:, cols],
                                    OP.mult,
                                )
                                eng.tensor_tensor(
                                    ts[:, cols], ts[:, cols], t1[:], OP.add
                                )
                            ts_l.append(ts)

                        bias_rb(0)
                        if DEBUG_TAPS:
                            nc.sync.dma_start(dbg["dkt"][:, :, :], kt_t[:].bitcast(F32))
                            nc.sync.dma_start(dbg["dqt2"][:, :, :], qt2_t[:].bitcast(F32))
                            nc.sync.dma_start(dbg["da12"][:, :, :], a12_t[:])
                            nc.sync.dma_start(dbg["db1"][:, :], b1bc_t[:])
                            nc.sync.dma_start(dbg["db2"][:, :], b2bc_t[:])
                            nc.sync.dma_start(dbg["dts"][:, :], ts_l[0][:])

                        tail_prev = None
                        for rb in range(RB):
                            ts = ts_l[rb]
                            expa = wp.tile([128, N], BF16, tag="expa", bufs=2)
                            zacc = wp.tile([128, 8], F32, tag="zacc", bufs=2)
                            wscr = wp.tile([128, N], BF16, tag="wscr", bufs=1)
                            sm = wp.tile([128, 4], F32, tag="sm", bufs=2)
                            pxa = ps2.tile([128, FEAT], F32, tag="kb2", bufs=2, name="pxa")
                            pxb = ps2.tile([128, FEAT], F32, tag="kb2", bufs=2, name="pxb")
                            abank_of = {}

                            def a_pair(p):
                                # chunks 2p, 2p+1: bias seed + 2 qk matmuls
                                n0, n1 = 2 * p, 2 * p + 1
                                ab = [
                                    ps2.tile([128, 512], F32, tag=f"kb{i}",
                                             bufs=2, name=f"ab{i}")
                                    for i in range(2)
                                ]
                                abank_of[n0] = ab[0]
                                abank_of[n1] = ab[1]
                                # qk first so A matmuls never wait on the
                                # DVE bias chain; the bias seed lands last
                                for hg in range(2):
                                    for i, n in ((0, n0), (1, n1)):
                                        nc.tensor.matmul(
                                            ab[i][:],
                                            qt2_t[:, hg, rb * 128 : (rb + 1) * 128],
                                            kt_t[:, hg, n * 512 : (n + 1) * 512],
                                            start=(hg == 0), stop=False,
                                            skip_group_check=True,
                                        )
                                for i, n in ((0, n0), (1, n1)):
                                    nc.tensor.matmul(
                                        ab[i][:], ident_t[:],
                                        ts[:, n * 512 : (n + 1) * 512],
                                        start=False, stop=True,
                                        skip_group_check=True,
                                    )

                            def exp_chunk(n):
                                nc.scalar.activation(
                                    expa[:, n * 512 : (n + 1) * 512],
                                    abank_of.pop(n)[:],
                                    AF.Exp,
                                    scale=float(SCALE),
                                    accum_out=zacc[:, n : n + 1],
                                )

                            u1c = wp.tile([128, 2], F32, tag="u1c", bufs=2)
                            u2c = wp.tile([128, 2], F32, tag="u2c", bufs=2)

                            def u_half(h):
                                # fused multiply+row-accumulate on DVE (the
                                # plain TensorReduce runs at 1x, this is the
                                # same cost with the reduce folded in)
                                cols = slice(h * 2048, (h + 1) * 2048)
                                nc.vector.scalar_tensor_tensor(
                                    wscr[:, cols], mu_l[rb][:, cols], 1.0,
                                    expa[:, cols], OP.mult, OP.mult,
                                    accum_out=u1c[:, h : h + 1],
                                )
                                nc.vector.scalar_tensor_tensor(
                                    wscr[:, cols], sd_l[rb][:, cols], 1.0,
                                    expa[:, cols], OP.mult, OP.mult,
                                    accum_out=u2c[:, h : h + 1],
                                )

                            pt_of = {}

                            def tp_t(g):
                                # transpose P chunk g into PT; drain on DVE
                                # for early groups (ACT still busy with exps)
                                # and on ACT for late ones (DVE busy with u)
                                tp = ps2.tile([128, 1024], BF16, tag="kb3", bufs=2, name="tp")
                                pt = wp.tile([128, 512], BF16, tag="pt", bufs=2)
                                pt_of[g] = pt
                                for j in range(4):
                                    c = 4 * g + j
                                    nc.tensor.transpose(
                                        tp[:, j * 128 : (j + 1) * 128],
                                        expa[:, c * 128 : (c + 1) * 128],
                                        ident_t[:],
                                    )
                                nc.scalar.copy(pt[:], tp[:, 0:512])

                            def tp_x(g):
                                # 4 PX accumulation matmuls vs x tiles; PX is
                                # split into two half-accumulations so the
                                # M-tail for the first half overlaps groups
                                # 4-7 instead of serializing after group 7
                                px = pxa if g < 4 else pxb
                                pt = pt_of.pop(g)
                                for j in range(4):
                                    c = 4 * g + j
                                    nc.tensor.matmul(
                                        px[:],
                                        pt[:, j * 128 : (j + 1) * 128],
                                        xb_t[:, c, :],
                                        start=(c % 16 == 0),
                                        stop=(c % 16 == 15),
                                        skip_group_check=True,
                                    )

                            def half_tail(px, mm, first):
                                pxs = wp.tile([128, FEAT], BF16, tag="pxs",
                                              bufs=2)
                                nc.scalar.copy(pxs[:], px[:])
                                tp2 = ps2.tile([128, 1024], BF16, tag="kb3",
                                               bufs=2, name="tp2")
                                pxt = wp.tile([128, 512], BF16, tag="pxt",
                                              bufs=2)
                                for j in range(4):
                                    nc.tensor.transpose(
                                        tp2[:, j * 128 : (j + 1) * 128],
                                        pxs[:, j * 128 : (j + 1) * 128],
                                        ident_t[:],
                                    )
                                nc.scalar.copy(pxt[:], tp2[:, 0:512])
                                for j in range(4):
                                    nc.tensor.matmul(
                                        mm[:],
                                        pxt[:, j * 128 : (j + 1) * 128],
                                        wv_t[:, j, :],
                                        start=(first and j == 0),
                                        stop=((not first) and j == 3),
                                        skip_group_check=True,
                                    )

                            def mk_tail(rb, pxb, mm, zacc, sm, expa):
                                def tail():
                                    # Z and u scaling
                                    nc.vector.tensor_reduce(
                                        sm[:, 0:1], zacc[:], AX.X, OP.add
                                    )
                                    nc.vector.reciprocal(sm[:, 1:2], sm[:, 0:1])
                                    nc.vector.tensor_scalar(
                                        sm[:, 2:3], sm[:, 2:3], sm[:, 1:2],
                                        None, OP.mult,
                                    )
                                    nc.vector.tensor_scalar(
                                        sm[:, 3:4], sm[:, 3:4], sm[:, 1:2],
                                        None, OP.mult,
                                    )
                                    # M tail, second PX half
                                    half_tail(pxb, mm, first=False)
                                    # mf = mm*rz + u1*rz*Ev1 + u2*rz*dEv
                                    mcp = wp.tile(
                                        [128, FEAT], F32, tag="mcp", bufs=1
                                    )
                                    mf = wp.tile([128, FEAT], F32, tag="mf",
                                                 bufs=1)
                                    nc.scalar.activation(
                                        mcp[:], mm[:], AF.Identity,
                                        scale=sm[:, 1:2],
                                    )
                                    nc.vector.scalar_tensor_tensor(
                                        mcp[:], ev1bc_t[:], sm[:, 2:3], mcp[:],
                                        OP.mult, OP.add,
                                    )
                                    nc.vector.scalar_tensor_tensor(
                                        mf[:], devbc_t[:], sm[:, 3:4], mcp[:],
                                        OP.mult, OP.add,
                                    )
                                    nc.sync.dma_start(
                                        mout[rb * 128 : (rb + 1) * 128, :], mf[:]
                                    )
                                    if DEBUG_TAPS and rb == 0:
                                        nc.sync.dma_start(
                                            dbg["dexpa"][:, :], expa[:]
                                        )
                                        nc.sync.dma_start(dbg["dpxs"][:, :], pxs[:])
                                        nc.sync.dma_start(dbg["dsm"][:, :], sm[:])

                                return tail

                            # pipelined emission: transpose groups lag the A
                            # pairs by two pairs, PX matmuls lag their
                            # transposes by one group (hides the Pool copy),
                            # and the previous rb's M-tail fills this rb's
                            # A-pair window
                            a_pair(0)
                            a_pair(1)
                            if tail_prev is not None:
                                tail_prev()
                            exp_chunk(0)
                            exp_chunk(1)
                            exp_chunk(2)
                            exp_chunk(3)
                            u_half(0)
                            a_pair(2)
                            exp_chunk(4)
                            exp_chunk(5)
                            tp_t(0)
                            tp_t(1)
                            tp_x(0)
                            a_pair(3)
                            exp_chunk(6)
                            exp_chunk(7)
                            tp_t(2)
                            tp_x(1)
                            tp_t(3)
                            tp_x(2)
                            u_half(1)
                            # prefetch + bias for rb+1 before this rb's tail
                            if rb + 2 < RB:
                                load_rb(rb + 2)
                            if rb + 1 < RB:
                                bias_rb(rb + 1)
                            tp_t(4)
                            tp_x(3)
                            tp_t(5)
                            tp_x(4)
                            mm = ps2.tile([128, FEAT], F32, tag="kb2",
                                          bufs=2, name="mm")
                            half_tail(pxa, mm, first=True)
                            tp_t(6)
                            tp_x(5)
                            tp_t(7)
                            tp_x(6)
                            tp_x(7)
                            nc.vector.tensor_reduce(
                                sm[:, 2:3], u1c[:], AX.X, OP.add
                            )
                            nc.vector.tensor_reduce(
                                sm[:, 3:4], u2c[:], AX.X, OP.add
                            )
                            tail_prev = mk_tail(rb, pxb, mm, zacc, sm, expa)
                        tail_prev()

            if reps > 1:
                with tc.For_i(0, reps, 1):
                    _body()
            else:
                _body()

    nc.compile()
    return nc


def _prep(inputs):
    x = np.asarray(inputs["x"], np.float32)
    ei = np.asarray(inputs["edge_index"]).astype(np.int64)
    ea = np.asarray(inputs["edge_attr"], np.float32).reshape(-1)
    Wk = np.asarray(inputs["Wk"], np.float32)
    Wq = np.asarray(inputs["Wq"], np.float32)
    Wv = np.asarray(inputs["Wv"], np.float32)
    Ek = np.asarray(inputs["Ek"], np.float32)
    Eq = np.asarray(inputs["Eq"], np.float32)
    Ev = np.asarray(inputs["Ev"], np.float32)
    ib = float(np.asarray(inputs["idx_bias"]).reshape(()))
    im = float(np.asarray(inputs["idx_mult"]).reshape(()))

    assert BF16_NP is not None, "ml_dtypes needed"
    bf = BF16_NP

    src, dst = ei[0], ei[1]
    s_all = 1.0 / (1.0 + np.exp(-(ea - ib) * im))
    # dense per-(core, row-block) multiplicity and sigmoid-sum matrices
    flat = src * N + dst
    mu_f = np.zeros(N * N, np.float32)
    sd_f = np.zeros(N * N, np.float32)
    np.add.at(mu_f, flat, 1.0)
    np.add.at(sd_f, flat, s_all)
    mu_d = mu_f.reshape(NCORES, RB, 128, N).astype(bf)
    sd_d = sd_f.reshape(NCORES, RB, 128, N).astype(bf)

    qk = bf if QK_BF16 else np.float32
    xT = np.ascontiguousarray(x.T)  # [FEAT, N]
    shared = {
        "xt": np.ascontiguousarray(xT.reshape(4, 128, N).astype(qk)),
        "xb": np.ascontiguousarray(x.reshape(32, 128, FEAT).astype(bf)),
        "wk": np.ascontiguousarray(Wk.reshape(4, 128, HID).astype(qk)),
        "wq2": np.ascontiguousarray((2.0 * Wq).reshape(4, 128, HID).astype(qk)),
        "wv": np.ascontiguousarray(Wv.reshape(4, 128, FEAT).astype(bf)),
        "wba": np.ascontiguousarray(
            (Wq @ np.stack([Ek[1], Ek[0] - Ek[1]], axis=1)).reshape(4, 128, 2).astype(qk)
        ),
        "wbq": np.ascontiguousarray(
            (Wk @ np.stack([Eq[1], Eq[0] - Eq[1]], axis=1)).reshape(4, 128, 2).astype(qk)
        ),
        "ev1bc": np.ascontiguousarray(
            np.broadcast_to(Ev[1], (128, FEAT)).astype(bf)
        ),
        "devbc": np.ascontiguousarray(
            np.broadcast_to(Ev[0] - Ev[1], (128, FEAT)).astype(bf)
        ),
        "ident": np.eye(128, dtype=np.float32).astype(bf),
    }
    in_maps = []
    for cc in range(NCORES):
        m = dict(shared)
        m["xtloc"] = np.ascontiguousarray(
            xT[:, cc * RPC : (cc + 1) * RPC].reshape(4, 128, RPC).astype(qk)
        )
        m["mu"] = mu_d[cc]
        m["sd"] = sd_d[cc]
        in_maps.append(m)
    return in_maps


def get_nc(reps=1):
    if reps not in _CACHE:
        _CACHE[reps] = _build_nc(reps)
    return _CACHE[reps]


def kernel(**inputs) -> np.ndarray:
    nc = get_nc()
    in_maps = _prep(inputs)
    res = run_bass_kernel_spmd(nc, in_maps, list(range(NCORES)))
    return np.concatenate(
        [res.results[cc]["mloc"] for cc in range(NCORES)], axis=0
    ).astype(np.float32)


# revision 73
# speedup vs baseline: 1.0262x; 1.0247x over previous
"""Trainium2 Bass kernel for nn_DistanceAwareSelfAttentionHead.

Math (reference):
    s  = sigmoid((edge_attr - ib) * im)                    [E]
    rk = Ek1 + s*dEk ; rq = Eq1 + s*dEq ; rv = Ev1 + s*dEv (per edge, rank-1 in s)
    k = x@Wk ; q = x@Wq ; v = x@Wv
    A  = 2 q k^T ; A[src,dst] += q[src].rk + k[dst].rq     (duplicate edges summed)
    P  = softmax(A / sqrt(512))
    M  = P v + segsum(P[src,dst] * rv, src)

Identities:
    summed bias of a cell = mu*(a1+b1) + Sd*(a2+b2)
        mu = cell multiplicity, Sd = sum of sigmoids over the cell's edges
        a1 = q.Ek1, a2 = q.dEk (per row), b1 = k.Eq1, b2 = k.dEq (per col)
    segsum(P*rv) row r = u1[r]*Ev1 + u2[r]*dEv, u1 = sum mu*P, u2 = sum Sd*P
    M = P@v = (P@x)@Wv  (v never materialized)
    softmax without max-subtraction (logits bounded), normalize at the end.

Sharding: rows of A/q/M split across 8 cores (512 rows each); k and the
x-resident matmuls replicated per core (no collectives). Host precomputes
dense per-(core,row-block) mu and Sd matrices (index/attr transforms only).

Per-core schedule (engines balanced; one PSUM pool spans both phases so
there is no drain barrier at the boundary; bulk DMA batched because every
dma_start costs ~625ns of serialized HWDGE descriptor generation):
  phase 1: kT (bf16 in, f32r out, weight-stationary half-sweeps over the
           8 PSUM banks), qT2, a12, b-rows via eqb^T@kT per half, then
           b-tables broadcast to 128 partitions by a DMA bounce through
           DRAM on the ACT HWDGE queue (no engine time, jumps the SP queue).
  phase 2 per row-block rb (software-pipelined, emission order = per-engine
           execution order):
           bias ts = (b1bc+a1)*mu + (b2bc+a2)*Sd via plain tensor_scalar/
           tensor_tensor (DVE 2x fast mode; the sd-mult and final add ride
           the Pool engine except for rb0 whose bias gates the pipeline);
           A chunk pairs: 2 q.k matmuls first (never wait on bias), ident-
           matmul seeds the bias last; exp on ACT with accum_out -> Z;
           u1/u2 = accum(mu*P), accum(Sd*P) fused stt halves on DVE;
           P^T tiles via PE transposes (ACT drains; PX matmuls lag one
           group to hide the copy); PX = P@x accumulated in PSUM;
           M = (PX)^T @ Wv; each rb's M-tail is deferred into the next
           rb's A-pair window so its latency overlaps.
"""

import sys

if "/opt/trn_rl_repo" not in sys.path:
    sys.path.insert(0, "/opt/trn_rl_repo")

from contextlib import ExitStack

import numpy as np

import concourse.bacc as bacc
import concourse.mybir as mybir
import concourse.tile as tile
from concourse.bass_utils import run_bass_kernel_spmd

try:
    import ml_dtypes

    BF16_NP = ml_dtypes.bfloat16
except Exception:  # pragma: no cover
    BF16_NP = None

# ---- problem constants (hardcoded per the harness contract) ----
N = 4096
FEAT = 512
HID = 256
NCORES = 8
RPC = N // NCORES  # 512 rows per core
RB = RPC // 128  # 4 row blocks
SCALE = 1.0 / np.sqrt(np.float32(FEAT))

F32 = mybir.dt.float32
F32R = mybir.dt.float32r
BF16 = mybir.dt.bfloat16

AF = mybir.ActivationFunctionType
OP = mybir.AluOpType
AX = mybir.AxisListType

_CACHE = {}
DEBUG_TAPS = False
QK_BF16 = True  # bf16 for the q/k pipeline (fallback: f32r)


def _build_nc(reps=1):
    nc = bacc.Bacc(
        "TRN2",
        target_bir_lowering=False,
        debug=False,
        enable_asserts=False,
        num_devices=NCORES,
    )
    d = {}

    def din(name, shape, dtype=F32):
        d[name] = nc.dram_tensor(name, shape, dtype, kind="ExternalInput").ap()

    QK = BF16 if QK_BF16 else F32R
    din("xt", [4, 128, N], QK)  # x^T, feat-major chunks
    din("xtloc", [4, 128, RPC], QK)  # local columns of x^T
    din("xb", [32, 128, FEAT], BF16)  # x node-major tiles (bf16)
    din("wk", [4, 128, HID], QK)
    din("wq2", [4, 128, HID], QK)  # 2*Wq
    din("wv", [4, 128, FEAT], BF16)  # Wv feat-in chunks
    din("wba", [4, 128, 2], QK)  # Wq @ [Ek1 | dEk]  (feat chunks)
    din("wbq", [4, 128, 2], QK)  # Wk @ [Eq1 | dEq]  (feat chunks)
    din("ev1bc", [128, FEAT], BF16)  # Ev1 broadcast
    din("devbc", [128, FEAT], BF16)  # dEv broadcast
    din("ident", [128, 128], BF16)
    din("mu", [RB, 128, N], BF16)  # dense cell multiplicity, 0 off-cell
    din("sd", [RB, 128, N], BF16)  # dense per-cell sum of sigmoids
    mout = nc.dram_tensor("mloc", [RPC, FEAT], F32, kind="ExternalOutput").ap()
    # DRAM bounce for partition-broadcasting the b-row tables
    brow_d = nc.dram_tensor("browd", [2, N], BF16, kind="Internal").ap()
    dbg = {}
    if DEBUG_TAPS:
        for nm, shape, dt in [
            ("dkt", [128, 2, N], F32),
            ("dqt2", [128, 2, RPC], F32),
            ("da12", [128, RB, 2], F32),
            ("db1", [128, N], BF16),
            ("db2", [128, N], BF16),
            ("dts", [128, N], BF16),
            ("dexpa", [128, N], BF16),
            ("dpxs", [128, FEAT], BF16),
            ("dsm", [128, 4], F32),
        ]:
            dbg[nm] = nc.dram_tensor(nm, shape, dt, kind="ExternalOutput").ap()

    with tile.TileContext(nc) as tc:
        with ExitStack() as ctx:
            cpool = ctx.enter_context(tc.tile_pool(name="consts", bufs=1))
            wk_t = cpool.tile([128, 4, HID], QK)
            wq2_t = cpool.tile([128, 4, HID], QK)
            wv_t = cpool.tile([128, 4, FEAT], BF16)
            wba_t = cpool.tile([128, 4, 2], QK)
            wbq_t = cpool.tile([128, 4, 2], QK)
            ident_t = cpool.tile([128, 128], BF16)
            ev1bc_t = cpool.tile([128, FEAT], BF16)
            devbc_t = cpool.tile([128, FEAT], BF16)
            xb_t = cpool.tile([128, 32, FEAT], BF16)

            # one DMA per tensor — each dma_start costs ~625ns of serialized
            # HWDGE descriptor generation, so small transfers must be batched.
            # Only wk gates the first matmuls; the rest go after xt (emitted
            # in _body) via this deferred hook.
            nc.sync.dma_start(wk_t[:], d["wk"][:].rearrange("c p h -> p c h"))

            def load_consts():
                nc.sync.dma_start(
                    wq2_t[:], d["wq2"][:].rearrange("c p h -> p c h")
                )
                nc.sync.dma_start(ident_t[:], d["ident"][:])
                nc.sync.dma_start(
                    wba_t[:], d["wba"][:].rearrange("c p h -> p c h")
                )
                nc.sync.dma_start(wbq_t[:], d["wbq"][:].rearrange("c p h -> p c h"))
                nc.sync.dma_start(wv_t[:], d["wv"][:].rearrange("c p h -> p c h"))
                nc.sync.dma_start(ev1bc_t[:], d["ev1bc"][:])
                nc.sync.dma_start(devbc_t[:], d["devbc"][:])

            def _body():
                with tc.tile_pool(name="mid", bufs=1) as pmid:
                    kt_t = pmid.tile([128, 2, N], F32R)
                    qt2_t = pmid.tile([128, 2, RPC], F32R)
                    b1bc_t = pmid.tile([128, N], BF16)
                    b2bc_t = pmid.tile([128, N], BF16)
                    a12_t = pmid.tile([128, RB, 2], F32)
                    brow_t = pmid.tile([2, N], BF16)

                    dmat_ctx = ExitStack()
                    dmp = dmat_ctx.enter_context(tc.tile_pool(name="dmat", bufs=2))
                    mu_l, sd_l = [], []

                    def load_rb(rb):
                        m = dmp.tile([128, N], BF16, tag="mu", name="mu")
                        s = dmp.tile([128, N], BF16, tag="sd", name="sd")
                        nc.sync.dma_start(m[:], d["mu"][rb])
                        nc.sync.dma_start(s[:], d["sd"][rb])
                        mu_l.append(m)
                        sd_l.append(s)

                    ps1 = dmat_ctx.enter_context(
                        tc.tile_pool(name="ps", bufs=1, space="PSUM")
                    )
                    ps2 = ps1
                    # ---------- phase 1: kT, qT2, a12, b tables ---------------
                    with tc.tile_pool(name="ph1", bufs=1) as p1:
                        xt_t = p1.tile([128, 4, N], QK)
                        xtloc_t = p1.tile([128, 4, RPC], QK)
                        # DMA order = criticality: xt feeds the k matmuls
                        # that gate everything else; h-outer so the first
                        # column-half of every kc chunk lands first
                        # split xt across the SP and ACT HWDGE queues so
                        # descriptor generation runs in parallel
                        for kc in range(4):
                            eng = nc.sync if kc % 2 == 0 else nc.scalar
                            eng.dma_start(
                                xt_t[:, kc, 0:2048], d["xt"][kc][:, 0:2048]
                            )
                        nc.sync.dma_start(
                            xtloc_t[:], d["xtloc"][:].rearrange("c p f -> p c f")
                        )
                        for kc in range(4):
                            eng = nc.sync if kc % 2 == 0 else nc.scalar
                            eng.dma_start(
                                xt_t[:, kc, 2048:N], d["xt"][kc][:, 2048:N]
                            )
                        load_consts()
                        load_rb(0)
                        # x node-major tiles for the P@x matmuls (bf16)
                        for g in range(8):
                            nc.sync.dma_start(
                                xb_t[:, 4 * g : 4 * g + 4, :],
                                d["xb"][4 * g : 4 * g + 4].rearrange(
                                    "t p f -> p t f"
                                ),
                            )
                        load_rb(1)

                        def k_half(half):
                            for hg in range(2):
                                # weight-stationary half-sweep over two wide
                                # (2-bank) psum tiles, accum over kc
                                kbank = [
                                    ps1.tile(
                                        [128, 1024], F32, tag="kb0",
                                        name=f"kw{i}", bufs=2,
                                    )
                                    for i in range(2)
                                ]
                                for kc in range(4):
                                    for i in range(4):
                                        n = 4 * half + i
                                        nc.tensor.matmul(
                                            kbank[i // 2][
                                                :, (i % 2) * 512 : (i % 2 + 1) * 512
                                            ],
                                            wk_t[:, kc, hg * 128 : (hg + 1) * 128],
                                            xt_t[:, kc, n * 512 : (n + 1) * 512],
                                            start=(kc == 0),
                                            stop=(kc == 3),
                                        )
                                # drain to kt (f32r), wide, split engines
                                n0 = 4 * half
                                nc.scalar.copy(
                                    kt_t[:, hg, n0 * 512 : (n0 + 2) * 512],
                                    kbank[0][:],
                                )
                                nc.vector.tensor_copy(
                                    kt_t[:, hg, (n0 + 2) * 512 : (n0 + 4) * 512],
                                    kbank[1][:],
                                )
                            # b rows straight from x^T (host premultiplied
                            # wbq = Wk @ [Eq1|dEq])
                            for i in range(4):
                                n = 4 * half + i
                                bps = ps1.tile(
                                    [128, 512], F32, tag="kb2", name="bps",
                                    bufs=2,
                                )
                                for kc in range(4):
                                    nc.tensor.matmul(
                                        bps[0:2, :],
                                        wbq_t[:, kc, :],
                                        xt_t[:, kc, n * 512 : (n + 1) * 512],
                                        start=(kc == 0),
                                        stop=(kc == 3),
                                    )
                                nc.vector.tensor_copy(
                                    brow_t[:, n * 512 : (n + 1) * 512], bps[0:2, :]
                                )
                            cols = slice(half * 2048, (half + 1) * 2048)
                            nc.scalar.dma_start(brow_d[:, cols], brow_t[:, cols])
                            nc.scalar.dma_start(
                                b1bc_t[:, cols],
                                brow_d[0:1, cols].broadcast_to([128, 2048]),
                            )
                            nc.scalar.dma_start(
                                b2bc_t[:, cols],
                                brow_d[1:2, cols].broadcast_to([128, 2048]),
                            )

                        def q_and_a12():
                            # qT2 (2*q, hid-major, local rows)
                            for hg in range(2):
                                qps = ps1.tile(
                                    [128, 512], F32, tag="kb2", name="qps", bufs=2
                                )
                                for kc in range(4):
                                    nc.tensor.matmul(
                                        qps[:],
                                        wq2_t[:, kc, hg * 128 : (hg + 1) * 128],
                                        xtloc_t[:, kc, :],
                                        start=(kc == 0),
                                        stop=(kc == 3),
                                    )
                                nc.vector.tensor_copy(qt2_t[:, hg, :], qps[:])
                            # a12: per-row (a1|a2) = x_loc @ (Wq [Ek1|dEk])
                            for rb in range(RB):
                                aps = ps1.tile(
                                    [128, 512], F32, tag="kb2", name="aps", bufs=2
                                )
                                for kc in range(4):
                                    nc.tensor.matmul(
                                        aps[:, 0:2],
                                        xtloc_t[:, kc, rb * 128 : (rb + 1) * 128],
                                        wba_t[:, kc, :],
                                        start=(kc == 0),
                                        stop=(kc == 3),
                                    )
                                nc.vector.tensor_copy(a12_t[:, rb, :], aps[:, 0:2])

                        k_half(0)
                        q_and_a12()
                        k_half(1)

                    # ---------- phase 2: bias + A + exp + PX + M --------------
                    with dmat_ctx, tc.tile_pool(name="work", bufs=1) as wp:

                        ts_l = []

                        def bias_rb(rb):
                            # ts = (b1bc + a1)*mu + (b2bc + a2)*sd, emitted in
                            # column halves so rb0's first A-chunk seeds only
                            # wait on the first half of the table bounce.
                            # plain tensor_scalar / tensor_tensor get the DVE
                            # 2x fast mode; scalar_tensor_tensor does not.
                            # rb0 runs entirely on DVE: its bias gates the
                            # whole phase-2 pipeline, and the Pool engine's
                            # software ops are ~4x slower per element.
                            t1 = wp.tile([128, 2048], BF16, tag="t1", bufs=1)
                            ts = wp.tile([128, N], BF16, tag="ts", bufs=2)
                            eng = nc.vector if rb == 0 else nc.gpsimd
                            for cols in (slice(0, 2048), slice(2048, N)):
                                nc.vector.tensor_scalar(
                                    t1[:], b1bc_t[:, cols],
                                    a12_t[:, rb, 0:1], None, OP.add,
                                )
                                nc.vector.tensor_tensor(
                                    t1[:], t1[:], mu_l[rb][:, cols], OP.mult
                                )
                                nc.vector.tensor_scalar(
                                    ts[:, cols], b2bc_t[:, cols],
                                    a12_t[:, rb, 1:2], None, OP.add,
                                )
                                eng.tensor_tensor(
                                    ts[:, cols], ts[:, cols], sd_l[rb][:, cols],
                                    OP.mult,
                                )
                                eng.tensor_tensor(
                                    ts[:, cols], ts[:, cols], t1[:], OP.add
                                )
                            ts_l.append(ts)

                        bias_rb(0)
                        if DEBUG_TAPS:
                            nc.sync.dma_start(dbg["dkt"][:, :, :], kt_t[:].bitcast(F32))
                            nc.sync.dma_start(dbg["dqt2"][:, :, :], qt2_t[:].bitcast(F32))
                            nc.sync.dma_start(dbg["da12"][:, :, :], a12_t[:])
                            nc.sync.dma_start(dbg["db1"][:, :], b1bc_t[:])
                            nc.sync.dma_start(dbg["db2"][:, :], b2bc_t[:])
                            nc.sync.dma_start(dbg["dts"][:, :], ts_l[0][:])

                        tail_prev = None
                        for rb in range(RB):
                            ts = ts_l[rb]
                            expa = wp.tile([128, N], BF16, tag="expa", bufs=2)
                            zacc = wp.tile([128, 4], F32, tag="zacc", bufs=2)
                            wscr = wp.tile([128, N], BF16, tag="wscr", bufs=1)
                            sm = wp.tile([128, 4], F32, tag="sm", bufs=2)
                            pxa = ps2.tile([128, FEAT], F32, tag="kb2", bufs=2, name="pxa")
                            pxb = ps2.tile([128, FEAT], F32, tag="kb2", bufs=2, name="pxb")
                            abank_of = {}

                            def a_pair(p):
                                # chunks 2p, 2p+1 in one wide (2-bank) tile:
                                # qk first so A matmuls never wait on the
                                # DVE bias chain; the bias seed lands last
                                n0 = 2 * p
                                ab = ps2.tile([128, 1024], F32, tag="kb0",
                                              bufs=2, name="ab")
                                abank_of[p] = ab
                                for hg in range(2):
                                    for i in range(2):
                                        n = n0 + i
                                        nc.tensor.matmul(
                                            ab[:, i * 512 : (i + 1) * 512],
                                            qt2_t[:, hg, rb * 128 : (rb + 1) * 128],
                                            kt_t[:, hg, n * 512 : (n + 1) * 512],
                                            start=(hg == 0), stop=False,
                                            skip_group_check=True,
                                        )
                                for i in range(2):
                                    n = n0 + i
                                    nc.tensor.matmul(
                                        ab[:, i * 512 : (i + 1) * 512], ident_t[:],
                                        ts[:, n * 512 : (n + 1) * 512],
                                        start=False, stop=True,
                                        skip_group_check=True,
                                    )

                            def exp_pair(p):
                                # one exp over both chunks of the pair
                                nc.scalar.activation(
                                    expa[:, p * 1024 : (p + 1) * 1024],
                                    abank_of.pop(p)[:],
                                    AF.Exp,
                                    scale=float(SCALE),
                                    accum_out=zacc[:, p : p + 1],
                                )

                            u1c = wp.tile([128, 2], F32, tag="u1c", bufs=2)
                            u2c = wp.tile([128, 2], F32, tag="u2c", bufs=2)

                            def u_half(h):
                                # fused multiply+row-accumulate on DVE (the
                                # plain TensorReduce runs at 1x, this is the
                                # same cost with the reduce folded in)
                                cols = slice(h * 2048, (h + 1) * 2048)
                                nc.vector.scalar_tensor_tensor(
                                    wscr[:, cols], mu_l[rb][:, cols], 1.0,
                                    expa[:, cols], OP.mult, OP.mult,
                                    accum_out=u1c[:, h : h + 1],
                                )
                                nc.vector.scalar_tensor_tensor(
                                    wscr[:, cols], sd_l[rb][:, cols], 1.0,
                                    expa[:, cols], OP.mult, OP.mult,
                                    accum_out=u2c[:, h : h + 1],
                                )

                            pt_of = {}

                            def tp_t(g):
                                # transpose P chunk g into PT; drain on DVE
                                # for early groups (ACT still busy with exps)
                                # and on ACT for late ones (DVE busy with u)
                                tp = ps2.tile([128, 1024], BF16, tag="kb1", bufs=2, name="tp")
                                pt = wp.tile([128, 512], BF16, tag="pt", bufs=2)
                                pt_of[g] = pt
                                for j in range(4):
                                    c = 4 * g + j
                                    nc.tensor.transpose(
                                        tp[:, j * 128 : (j + 1) * 128],
                                        expa[:, c * 128 : (c + 1) * 128],
                                        ident_t[:],
                                    )
                                nc.scalar.copy(pt[:], tp[:, 0:512])

                            def tp_x(g):
                                # 4 PX accumulation matmuls vs x tiles; PX is
                                # split into two half-accumulations so the
                                # M-tail for the first half overlaps groups
                                # 4-7 instead of serializing after group 7
                                px = pxa if g < 4 else pxb
                                pt = pt_of.pop(g)
                                for j in range(4):
                                    c = 4 * g + j
                                    nc.tensor.matmul(
                                        px[:],
                                        pt[:, j * 128 : (j + 1) * 128],
                                        xb_t[:, c, :],
                                        start=(c % 16 == 0),
                                        stop=(c % 16 == 15),
                                        skip_group_check=True,
                                    )

                            def half_tail(px, mm, first):
                                pxs = wp.tile([128, FEAT], BF16, tag="pxs",
                                              bufs=2)
                                nc.scalar.copy(pxs[:], px[:])
                                tp2 = ps2.tile([128, 1024], BF16, tag="kb1",
                                               bufs=2, name="tp2")
                                pxt = wp.tile([128, 512], BF16, tag="pxt",
                                              bufs=2)
                                for j in range(4):
                                    nc.tensor.transpose(
                                        tp2[:, j * 128 : (j + 1) * 128],
                                        pxs[:, j * 128 : (j + 1) * 128],
                                        ident_t[:],
                                    )
                                nc.scalar.copy(pxt[:], tp2[:, 0:512])
                                for j in range(4):
                                    nc.tensor.matmul(
                                        mm[:],
                                        pxt[:, j * 128 : (j + 1) * 128],
                                        wv_t[:, j, :],
                                        start=(first and j == 0),
                                        stop=((not first) and j == 3),
                                        skip_group_check=True,
                                    )

                            def mk_tail(rb, pxb, mm, zacc, sm, expa):
                                def tail():
                                    # Z and u scaling
                                    nc.vector.tensor_reduce(
                                        sm[:, 0:1], zacc[:], AX.X, OP.add
                                    )
                                    nc.vector.reciprocal(sm[:, 1:2], sm[:, 0:1])
                                    nc.vector.tensor_scalar(
                                        sm[:, 2:3], sm[:, 2:3], sm[:, 1:2],
                                        None, OP.mult,
                                    )
                                    nc.vector.tensor_scalar(
                                        sm[:, 3:4], sm[:, 3:4], sm[:, 1:2],
                                        None, OP.mult,
                                    )
                                    # M tail, second PX half
                                    half_tail(pxb, mm, first=False)
                                    # mf = mm*rz + u1*rz*Ev1 + u2*rz*dEv
                                    mcp = wp.tile(
                                        [128, FEAT], F32, tag="mcp", bufs=1
                                    )
                                    mf = wp.tile([128, FEAT], F32, tag="mf",
                                                 bufs=1)
                                    nc.scalar.activation(
                                        mcp[:], mm[:], AF.Identity,
                                        scale=sm[:, 1:2],
                                    )
                                    nc.vector.scalar_tensor_tensor(
                                        mcp[:], ev1bc_t[:], sm[:, 2:3], mcp[:],
                                        OP.mult, OP.add,
                                    )
                                    nc.vector.scalar_tensor_tensor(
                                        mf[:], devbc_t[:], sm[:, 3:4], mcp[:],
                                        OP.mult, OP.add,
                                    )
                                    nc.sync.dma_start(
                                        mout[rb * 128 : (rb + 1) * 128, :], mf[:]
                                    )
                                    if DEBUG_TAPS and rb == 0:
                                        nc.sync.dma_start(
                                            dbg["dexpa"][:, :], expa[:]
                                        )
                                        nc.sync.dma_start(dbg["dpxs"][:, :], pxs[:])
                                        nc.sync.dma_start(dbg["dsm"][:, :], sm[:])

                                return tail

                            # pipelined emission: transpose groups lag the A
                            # pairs by two pairs, PX matmuls lag their
                            # transposes by one group (hides the Pool copy),
                            # and the previous rb's M-tail fills this rb's
                            # A-pair window
                            a_pair(0)
                            a_pair(1)
                            if tail_prev is not None:
                                tail_prev()
                            exp_pair(0)
                            exp_pair(1)
                            u_half(0)
                            a_pair(2)
                            exp_pair(2)
                            tp_t(0)
                            tp_t(1)
                            tp_x(0)
                            a_pair(3)
                            exp_pair(3)
                            tp_t(2)
                            tp_x(1)
                            tp_t(3)
                            tp_x(2)
                            u_half(1)
                            # prefetch + bias for rb+1 before this rb's tail
                            if rb + 2 < RB:
                                load_rb(rb + 2)
                            if rb + 1 < RB:
                                bias_rb(rb + 1)
                            tp_t(4)
                            tp_x(3)
                            tp_t(5)
                            tp_x(4)
                            mm = ps2.tile([128, FEAT], F32, tag="kb2",
                                          bufs=2, name="mm")
                            half_tail(pxa, mm, first=True)
                            tp_t(6)
                            tp_x(5)
                            tp_t(7)
                            tp_x(6)
                            tp_x(7)
                            nc.vector.tensor_reduce(
                                sm[:, 2:3], u1c[:], AX.X, OP.add
                            )
                            nc.vector.tensor_reduce(
                                sm[:, 3:4], u2c[:], AX.X, OP.add
                            )
                            tail_prev = mk_tail(rb, pxb, mm, zacc, sm, expa)
                        tail_prev()

            if reps > 1:
                with tc.For_i(0, reps, 1):
                    _body()
            else:
                _body()

    nc.compile()
    return nc


def _prep(inputs):
    x = np.asarray(inputs["x"], np.float32)
    ei = np.asarray(inputs["edge_index"]).astype(np.int64)
    ea = np.asarray(inputs["edge_attr"], np.float32).reshape(-1)
    Wk = np.asarray(inputs["Wk"], np.float32)
    Wq = np.asarray(inputs["Wq"], np.float32)
    Wv = np.asarray(inputs["Wv"], np.float32)
    Ek = np.asarray(inputs["Ek"], np.float32)
    Eq = np.asarray(inputs["Eq"], np.float32)
    Ev = np.asarray(inputs["Ev"], np.float32)
    ib = float(np.asarray(inputs["idx_bias"]).reshape(()))
    im = float(np.asarray(inputs["idx_mult"]).reshape(()))

    assert BF16_NP is not None, "ml_dtypes needed"
    bf = BF16_NP

    src, dst = ei[0], ei[1]
    s_all = 1.0 / (1.0 + np.exp(-(ea - ib) * im))
    # dense per-(core, row-block) multiplicity and sigmoid-sum matrices
    flat = src * N + dst
    mu_f = np.zeros(N * N, np.float32)
    sd_f = np.zeros(N * N, np.float32)
    np.add.at(mu_f, flat, 1.0)
    np.add.at(sd_f, flat, s_all)
    mu_d = mu_f.reshape(NCORES, RB, 128, N).astype(bf)
    sd_d = sd_f.reshape(NCORES, RB, 128, N).astype(bf)

    qk = bf if QK_BF16 else np.float32
    xT = np.ascontiguousarray(x.T)  # [FEAT, N]
    shared = {
        "xt": np.ascontiguousarray(xT.reshape(4, 128, N).astype(qk)),
        "xb": np.ascontiguousarray(x.reshape(32, 128, FEAT).astype(bf)),
        "wk": np.ascontiguousarray(Wk.reshape(4, 128, HID).astype(qk)),
        "wq2": np.ascontiguousarray((2.0 * Wq).reshape(4, 128, HID).astype(qk)),
        "wv": np.ascontiguousarray(Wv.reshape(4, 128, FEAT).astype(bf)),
        "wba": np.ascontiguousarray(
            (Wq @ np.stack([Ek[1], Ek[0] - Ek[1]], axis=1)).reshape(4, 128, 2).astype(qk)
        ),
        "wbq": np.ascontiguousarray(
            (Wk @ np.stack([Eq[1], Eq[0] - Eq[1]], axis=1)).reshape(4, 128, 2).astype(qk)
        ),
        "ev1bc": np.ascontiguousarray(
            np.broadcast_to(Ev[1], (128, FEAT)).astype(bf)
        ),
        "devbc": np.ascontiguousarray(
            np.broadcast_to(Ev[0] - Ev[1], (128, FEAT)).astype(bf)
        ),
        "ident": np.eye(128, dtype=np.float32).astype(bf),
    }
    in_maps = []
    for cc in range(NCORES):
        m = dict(shared)
        m["xtloc"] = np.ascontiguousarray(
            xT[:, cc * RPC : (cc + 1) * RPC].reshape(4, 128, RPC).astype(qk)
        )
        m["mu"] = mu_d[cc]
        m["sd"] = sd_d[cc]
        in_maps.append(m)
    return in_maps


def get_nc(reps=1):
    if reps not in _CACHE:
        _CACHE[reps] = _build_nc(reps)
    return _CACHE[reps]


def kernel(**inputs) -> np.ndarray:
    nc = get_nc()
    in_maps = _prep(inputs)
    res = run_bass_kernel_spmd(nc, in_maps, list(range(NCORES)))
    return np.concatenate(
        [res.results[cc]["mloc"] for cc in range(NCORES)], axis=0
    ).astype(np.float32)
